# revision 12
# baseline (speedup 1.0000x reference)
"""BatchMatchedMSELoss on 8 Trainium2 NeuronCores.

loss = mean(concat(row_min, col_min)) of the (B,B) pairwise-MSE matrix
  mse[i,j] = (||x_i||^2 + ||y_j||^2 - 2 x_i.y_j) / D,  B=8192, D=1024.

Sharding: input rows split across 8 cores (1024 rows each); every core
computes its (1024, 8192) tile of D*mse = sqx[i] + sqy[j] - 2*cross via
bf16 matmuls (fp32 PSUM accumulation), with the sq terms folded into the
contraction as extra K-rows (hi/lo bf16 split for accuracy). Row mins are
full per-core results; column partial mins are combined on the host
(cheap 8x8192 elementwise min) along with the final mean.
"""

import numpy as np

import concourse.bass as bass
import concourse.tile as tile
import concourse.mybir as mybir
import concourse.masks as masks
from concourse.bass import ts
from concourse.bass_utils import run_bass_kernel_spmd

FP32 = mybir.dt.float32
BF16 = mybir.dt.bfloat16
AL = mybir.AluOpType
AX = mybir.AxisListType
AF = mybir.ActivationFunctionType

B = 8192          # batch (rows of input and target)
D = 1024          # feature dim (contraction)
NCORES = 8
RPC = B // NCORES  # rows per core = 1024
P = 128
MT = RPC // P      # 8 row tiles per core
DT = D // P        # 8 contraction tiles
CHUNK = 1024       # column chunk
NCH = B // CHUNK   # 8 chunks
HALF = 512         # max moving free dim per matmul / one PSUM bank
BIG = 1.0e30


def _legalize_waits(nc, max_waits=1):
    """walrus codegen in this container rejects instructions carrying more
    than one sync-wait command. Split extra waits onto standalone
    EventSemaphore instructions (same engine, immediately before), which is
    exactly what engine.wait_ge() emits."""
    n = 0
    for f in nc.m.functions:
        for bb in f.blocks:
            insts = bb.instructions
            out = []
            for inst in insts:
                si = inst.sync_info
                if si is not None and si.on_wait and len(si.on_wait) > max_waits:
                    waits = list(si.on_wait)
                    extra, keep = waits[:-max_waits], waits[-max_waits:]
                    for w in extra:
                        n += 1
                        ev = mybir.InstEventSemaphore(
                            name=f"legwait-{n}-{inst.name}", ins=[], outs=[]
                        )
                        ev.engine = inst.engine
                        ev.sync_info = mybir.SyncInfo(on_wait=[w], on_update=[])
                        out.append(ev)
                    inst.sync_info = mybir.SyncInfo(
                        on_wait=keep, on_update=list(si.on_update)
                    )
                out.append(inst)
            bb.instructions = out
    return n


def _sq_row(nc, pool_small, dst_thin, sqcols, width, tag):
    """Turn per-partition sq columns [P, width/P] into the two bf16 hi/lo
    K-rows of dst_thin [2, width] (partition 0 = hi, partition 1 = lo)."""
    sqrow = pool_small.tile([1, width], FP32, tag=f"sqrow{tag}", name=f"sqrow{tag}")
    # per-column transpose DMAs (SWDGE: needs >2 sync waits):
    # sqrow[0, rt*P + p] = sqcols[p, rt]
    for rt in range(width // P):
        nc.gpsimd.dma_start(
            out=sqrow[0:1, ts(rt, P)], in_=sqcols[:, rt : rt + 1]
        )
    nc.vector.tensor_copy(dst_thin[0:1, :], sqrow[:])          # hi = bf16(sq)
    lo = pool_small.tile([1, width], BF16, tag=f"lo{tag}", name=f"lo{tag}")
    nc.vector.scalar_tensor_tensor(
        lo[:], dst_thin[0:1, :], -1.0, sqrow[:], op0=AL.mult, op1=AL.add
    )
    nc.gpsimd.dma_start(out=dst_thin[1:2, :], in_=lo[0:1, :])  # lo to partition 1


def build_bass(legalize: bool = True) -> bass.Bass:
    nc = bass.Bass()
    x = nc.dram_tensor("x", [RPC, D], FP32, kind="ExternalInput")
    y = nc.dram_tensor("y", [B, D], FP32, kind="ExternalInput")
    rowmin_d = nc.dram_tensor("rowmin", [P, MT], FP32, kind="ExternalOutput")
    colmin_d = nc.dram_tensor("colmin", [1, B], FP32, kind="ExternalOutput")

    with tile.TileContext(nc) as tc:
        with (
            tc.tile_pool(name="consts", bufs=1) as consts,
            tc.tile_pool(name="xstage", bufs=2) as xstage,
            tc.tile_pool(name="ystage", bufs=3) as ystage,
            tc.tile_pool(name="ybf", bufs=18) as ybfp,
            tc.tile_pool(name="yt", bufs=2) as ytp,
            tc.tile_pool(name="work", bufs=3) as work,
            tc.tile_pool(name="small", bufs=2) as small,
            tc.tile_pool(name="pmm", bufs=3, space=bass.MemorySpace.PSUM) as pmm,
            tc.tile_pool(name="ptp", bufs=2, space=bass.MemorySpace.PSUM) as ptp,
        ):
            identity = consts.tile([P, P], BF16)
            masks.make_identity(nc, identity[:])
            ones2 = consts.tile([2, P], BF16)
            nc.vector.memset(ones2[:], 1.0)
            ones_rhs = consts.tile([2, HALF], BF16)
            nc.vector.memset(ones_rhs[:], 1.0)
            # rows: [sqx_hi; sqx_lo] along i — extra K-rows paired with ones
            thinX = consts.tile([2, RPC], BF16)
            rowmin_ch = consts.tile([P, MT * NCH], FP32)
            rowmin_out = consts.tile([P, MT], FP32)
            XT = [
                consts.tile([P, RPC], BF16, tag=f"xt{d}", name=f"xt{d}")
                for d in range(DT)
            ]

            # ---- Phase A: X prep (transpose -2X to [d, i], sqx hi/lo rows) ----
            sqcolsX = consts.tile([P, MT], FP32)
            for mt in range(MT):
                xf = xstage.tile([P, D], FP32, tag="xf")
                nc.sync.dma_start(out=xf[:], in_=x[ts(mt, P), :])
                # square+row-sum on DVE so xf has single-engine readers
                # (HWDGE direct2d DMAs only support 2 sync waits)
                sqsc = work.tile([P, D], BF16, tag="sqsc")
                nc.vector.scalar_tensor_tensor(
                    sqsc[:], xf[:], 1.0, xf[:], op0=AL.mult, op1=AL.mult,
                    accum_out=sqcolsX[:, mt : mt + 1],
                )
                xb = xstage.tile([P, D], BF16, tag="xb")
                nc.vector.tensor_scalar_mul(xb[:], xf[:], -2.0)
                pt = ptp.tile([P, D], BF16, tag="pt")
                for dt in range(DT):
                    nc.tensor.transpose(pt[:, ts(dt, P)], xb[:, ts(dt, P)], identity[:])
                for dt in range(DT):
                    nc.scalar.copy(XT[dt][:, ts(mt, P)], pt[:, ts(dt, P)])
            _sq_row(nc, small, thinX, sqcolsX, RPC, "x")

            # ---- Phase B: stream column chunks of Y ----
            for ch in range(NCH):
                j0 = ch * CHUNK
                sqcolsY = small.tile([P, CHUNK // P], FP32, tag="sqcols")
                ybts = []
                for rt in range(CHUNK // P):
                    yf = ystage.tile([P, D], FP32, tag="yf")
                    nc.sync.dma_start(
                        out=yf[:], in_=y[j0 + rt * P : j0 + (rt + 1) * P, :]
                    )
                    # yf readers stay ACT-only (2-wait limit on its load DMA)
                    sqsc = work.tile([P, D], BF16, tag="sqsc")
                    nc.scalar.activation(
                        sqsc[:], yf[:], AF.Square, accum_out=sqcolsY[:, rt : rt + 1]
                    )
                    yb = ybfp.tile([P, D], BF16, tag="yb")
                    nc.scalar.copy(yb[:], yf[:])
                    ybts.append(yb)
                thinY = work.tile([2, CHUNK], BF16, tag="thinY")
                _sq_row(nc, small, thinY, sqcolsY, CHUNK, "y")

                yts = []
                for dt in range(DT):
                    ptt = ptp.tile([P, CHUNK], BF16, tag="pt")
                    for rt in range(CHUNK // P):
                        nc.tensor.transpose(
                            ptt[:, ts(rt, P)], ybts[rt][:, ts(dt, P)], identity[:]
                        )
                    ytile = ytp.tile([P, CHUNK], BF16, tag=f"yt{dt}", name=f"yt{dt}")
                    nc.scalar.copy(ytile[:], ptt[:])
                    yts.append(ytile)

                colmin = work.tile([P, CHUNK], FP32, tag="colmin")
                for m in range(MT):
                    ps = pmm.tile([P, CHUNK], FP32, tag="ps")
                    for h in range(2):
                        hs = slice(h * HALF, (h + 1) * HALF)
                        for dt in range(DT):
                            nc.tensor.matmul(
                                ps[:, hs],
                                XT[dt][:, ts(m, P)],
                                yts[dt][:, hs],
                                start=(dt == 0),
                                stop=False,
                            )
                        nc.tensor.matmul(
                            ps[:, hs], thinX[:, ts(m, P)], ones_rhs[:],
                            start=False, stop=False,
                        )
                        nc.tensor.matmul(
                            ps[:, hs], ones2[:], thinY[:, hs],
                            start=False, stop=True,
                        )
                    k = m * NCH + ch
                    nc.vector.tensor_reduce(
                        out=rowmin_ch[:, k : k + 1], in_=ps[:], axis=AX.X, op=AL.min
                    )
                    if m == 0:
                        nc.vector.tensor_copy(colmin[:], ps[:])
                    else:
                        nc.vector.scalar_tensor_tensor(
                            colmin[:], ps[:], 0.0, colmin[:],
                            op0=AL.bypass, op1=AL.min,
                        )

                # min across the 128 partitions: DMA-shift + vector-min tree
                s = 64
                while s >= 1:
                    tmp = work.tile([64, CHUNK], FP32, tag="tree")
                    nc.sync.dma_start(out=tmp[:s, :], in_=colmin[s : 2 * s, :])
                    nc.vector.tensor_tensor(
                        colmin[0:s, :], colmin[0:s, :], tmp[:s, :], AL.min
                    )
                    s //= 2
                nc.sync.dma_start(
                    out=colmin_d[0:1, j0 : j0 + CHUNK], in_=colmin[0:1, :]
                )

            for m in range(MT):
                nc.vector.tensor_reduce(
                    out=rowmin_out[:, m : m + 1],
                    in_=rowmin_ch[:, m * NCH : (m + 1) * NCH],
                    axis=AX.X,
                    op=AL.min,
                )
            nc.sync.dma_start(out=rowmin_d[:, :], in_=rowmin_out[:, :])
    if legalize:
        _legalize_waits(nc)
    return nc


_NC_CACHE = None


def _get_nc():
    global _NC_CACHE
    if _NC_CACHE is None:
        _NC_CACHE = build_bass()
    return _NC_CACHE


def kernel(input, target):
    X = np.ascontiguousarray(np.asarray(input, dtype=np.float32))
    Y = np.ascontiguousarray(np.asarray(target, dtype=np.float32))
    assert X.shape == (B, D) and Y.shape == (B, D)

    nc = _get_nc()
    in_maps = [
        {"x": X[c * RPC : (c + 1) * RPC], "y": Y} for c in range(NCORES)
    ]
    res = run_bass_kernel_spmd(nc, in_maps, core_ids=list(range(NCORES))).results

    # rowmin[p, m] on core c = min_j D*mse for global row c*RPC + m*P + p
    row_sum = np.float64(0.0)
    col_parts = []
    for r in res:
        row_sum += r["rowmin"].astype(np.float64).sum()
        col_parts.append(r["colmin"].reshape(B))
    col_min = np.min(np.stack(col_parts), axis=0).astype(np.float64)
    loss = (row_sum + col_min.sum()) / D / (2 * B)
    return np.asarray(loss, dtype=np.float32)
